# revision 20
# baseline (speedup 1.0000x reference)
"""Trainium2 Bass kernel for nn_CubicSpline (embedding_lookup-style affine map).

Reference computes, for t in [0,1):
    w[n,i] = 1 - |t[n] - i|          (i = 0..62)
    out    = w @ cp[:63]             ([N,63] @ [63,128])

For t in [0,1] the triangular weights collapse algebraically:
    out[n,:] = t[n] * A + B
    A = sum_{i=1}^{62} cp[i] - cp[0]
    B = cp[0] + sum_{i=1}^{62} (1-i) * cp[i]

The device only needs to materialize a rank-1 affine map -- purely memory
bound on the output write. The device emits float16 (l2 rel err ~2e-4,
dominated by fp16 rounding of the result; gate is 2e-2) and the host
upcasts to float32, halving HBM write traffic, which is the roofline.

The PE on this part runs at a fixed 1.2 GHz column rate (HAM never
unthrottles), i.e. ~0.85 us per N=512 matmul, so the PE streaming of
123k psum columns (~105 us) and the 32.3 MB output DMA (~95 us) are
co-critical. Layout / engine budget per core:

  * rows grouped 24 per q-column: per-core row = 24*Qg + 8*c + j
    (Qg q-column, c chunk 0..2, j phase 0..7). Host packs
    t_aug[j, c, Qg] = fp16 t phases, rows 8/9 = ones (for B_hi/B_lo).
  * PE tiles: one K=10 weight load (lhsT = t_aug[:, c, 128h:+128]) and
    two N=512 fp16 matmuls against a constant block-diagonal rhs
    [10, 1024] (A on the phase diagonal, B_hi/B_lo on the ones rows)
    -> PSUM holds t*A + B for 1024 rows as [128 x 1024].
  * PSUM -> SBUF fp32->fp16 copies: ScalarE (3 of 5 PE tiles per
    6-block), VectorE (2 of 5).
  * every 6th tile bypasses the PE entirely: VectorE computes it with 8
    scalar_tensor_tensor ops, out[q, w*128:+128] = A_rep * t_col[q] +
    B_rep (per-partition scalar = the t value of output row 24q+8c+w).
  * copies/STT fill [128, 6144] fp16 buffers (2 groups); each buffer is
    one contiguous 1.5 MB HBM write. Output DMAs are issued ONLY from
    SyncE (SP-HWDGE) and GpSimdE (SWDGE) so their sem waits never
    head-of-line-block the compute engines' queues; the two rings
    alternate so each ring's ~2us completion stall hides under the
    other's streaming. The two final groups go as ring-parallel singles
    to shorten the tail.
"""

import os
import sys
from contextlib import ExitStack

for _p in ("/opt/trn_rl_repo", "/root/.axon_site/_ro/trn_rl_repo"):
    if os.path.isdir(_p) and _p not in sys.path:
        sys.path.insert(0, _p)

import ml_dtypes
import numpy as np

import concourse.mybir as mybir
import concourse.tile as tile
from concourse import bacc
from concourse import bass_utils

N_TOTAL = 1_000_000
D = 128
NUM_CP = 64
N_CORES = 8

R = 8                    # phase rows per chunk (= rows per partition per psum tile)
G = 3                    # chunks (psum tiles) per output group
KC = R + 2               # contraction rows: 8 fp16 t phases + 2 ones rows
S = R // 4               # N=512 matmuls per psum tile
TILE_ROWS = 128 * R      # rows per psum tile
TILES = 123              # tiles per core
GROUPS = TILES // G      # output groups (768 KB each) per core
NPC = TILES * TILE_ROWS  # rows per core
NPAD = N_CORES * NPC     # padded rows total
QTOT = NPC // (R * G)    # q-columns per core
STT_MOD = 6              # every 6th tile computed on VectorE, not the PE

F32 = mybir.dt.float32
F16 = mybir.dt.float16
NPF16 = np.float16
NPBF16 = ml_dtypes.bfloat16


def stt_tiles():
    return [g for g in range(TILES) if g % STT_MOD == STT_MOD - 1]


# GpSimd supports neither PSUM reads nor TensorScalarPtr (walrus ISA
# check), so it is DMA-issue only; ScalarE takes 4 of 5 PE-tile copies
# and VectorE takes 1 copy + all STT tiles.


def build_body(tc, out_ap, t_aug_ap, rhs_ap, stt_ap, groups, qtot):
    """Tile-framework kernel body."""
    nc = tc.nc
    mult, add = mybir.AluOpType.mult, mybir.AluOpType.add
    sttset = set(stt_tiles())
    # single-group view: [groups, 128, 3072], group h / partition q / (w d)
    # -> row 24*(128h+q) + w', col d: contiguous 6 KB per partition.
    out_t1 = out_ap.rearrange("(h q w) d -> h q (w d)", q=128, w=R * G)
    # paired view for groups 1-2 and tripled view for groups 3..38: one
    # contiguous 1.5 / 2.25 MB write per dma_start.
    grows = TILE_ROWS * G
    out_t2 = out_ap[grows : 3 * grows, :].rearrange(
        "(P p q w) d -> P q p (w d)", p=2, q=128, w=R * G
    )
    ntrip = (groups - 5) // 3
    out_t3 = out_ap[3 * grows : (3 + 3 * ntrip) * grows, :].rearrange(
        "(P p q w) d -> P q p (w d)", p=3, q=128, w=R * G
    )

    with ExitStack() as ctx:
        psum_bufs = (16 * 1024) // (TILE_ROWS * 4)  # fill the 8 PSUM banks
        tpool = ctx.enter_context(tc.tile_pool(name="tpool", bufs=1))
        cpool = ctx.enter_context(tc.tile_pool(name="cpool", bufs=1))
        opool = ctx.enter_context(tc.tile_pool(name="opool", bufs=3))
        ppool = ctx.enter_context(
            tc.tile_pool(name="ppool", bufs=psum_bufs, space="PSUM")
        )

        # DMA-issue rings: engines with no compute work, so their
        # dma_start sem-waits cannot block copies.
        rings = [nc.sync, nc.gpsimd]

        # t_aug chunks: first small chunk lands fast for the first matmul.
        ngroups = qtot // 128
        bounds = [0, 6 * 128]
        take = (ngroups - 6 + 1) // 2
        bounds.append(bounds[1] + take * 128)
        bounds.append(ngroups * 128)
        t_tiles = []
        for c in range(len(bounds) - 1):
            lo, hi = bounds[c], bounds[c + 1]
            tt = tpool.tile([KC, G, hi - lo], F16, name=f"tch{c}", tag=f"tch{c}")
            rings[c % 2].dma_start(tt[:], t_aug_ap[:, :, lo:hi])
            t_tiles.append(tt)

        # constants ride the ACT HWDGE path (ScalarE is idle until the
        # first copy, and these are ready at kernel start).
        rhs_sb = cpool.tile([KC, S * 512], F16)
        for s in range(S):
            nc.scalar.dma_start(rhs_sb[:, 512 * s : 512 * (s + 1)], rhs_ap[s])
        nstt = len(sttset)
        ab_rep = cpool.tile([128, 2 * D], F16, name="ab_rep")
        nc.scalar.dma_start(ab_rep[:], stt_ap[:, : 2 * D])
        t_stt = cpool.tile([128, R * nstt], F16, name="t_stt")
        nc.scalar.dma_start(t_stt[:], stt_ap[:, 2 * D : 2 * D + R * nstt])
        a_rep = ab_rep[:, :D]
        b_rep = ab_rep[:, D : 2 * D]

        def lhsT_for(h, c):
            col = h * 128
            for i in range(len(bounds) - 1):
                if col < bounds[i + 1]:
                    off = col - bounds[i]
                    return t_tiles[i][:, c, off : off + 128]
            raise AssertionError

        # DMA chunks: [0] [1-2] [3-5 .. 36-38] [39] [40]
        chunks = [(0, 1), (1, 2)] + [(3 + 3 * p, 3) for p in range(ntrip)]
        chunks += [(groups - 2, 1), (groups - 1, 1)]

        nthst = 0  # running STT tile index
        ci = 0     # ring rotation index
        for start, glen in chunks:
            ob = opool.tile([128, 3 * G * TILE_ROWS], F16, name="ob")
            for gi in range(glen * G):
                g = start * G + gi
                h, c = divmod(g, G)
                osl = slice(gi * TILE_ROWS, (gi + 1) * TILE_ROWS)
                if g in sttset:
                    # per phase w, the rows 24*(128h+q) + 8c + w form one
                    # partition-aligned [128, 128] block:
                    # out = A_rep * t + B_rep (no PE, no PSUM).
                    eng = nc.vector
                    for w in range(R):
                        tcol = t_stt[:, nthst * R + w : nthst * R + w + 1]
                        dsl = slice(gi * TILE_ROWS + w * D,
                                    gi * TILE_ROWS + (w + 1) * D)
                        eng.scalar_tensor_tensor(
                            ob[:, dsl], a_rep, tcol, b_rep, mult, add
                        )
                    nthst += 1
                    continue
                psum = ppool.tile([128, TILE_ROWS], F32, name="psum")
                lhsT = lhsT_for(h, c)
                for s in range(S):
                    sl = slice(512 * s, 512 * (s + 1))
                    nc.tensor.matmul(
                        psum[:, sl], lhsT, rhs_sb[:, sl], start=True, stop=True
                    )
                if g % STT_MOD == 3:
                    nc.vector.tensor_copy(ob[:, osl], psum[:])
                else:
                    nc.scalar.copy(ob[:, osl], psum[:])
            if glen == 3:
                dst = out_t3[(start - 3) // 3]
                src = ob[:].rearrange("q (p f) -> q p f", p=3)
                rings[ci % 2].dma_start(dst, src)
                ci += 1
            elif glen == 2:
                dst = out_t2[0]
                src = ob[:, : 2 * G * TILE_ROWS].rearrange(
                    "q (p f) -> q p f", p=2
                )
                rings[ci % 2].dma_start(dst, src)
                ci += 1
            elif start == 0:
                rings[ci % 2].dma_start(out_t1[0], ob[:, : G * TILE_ROWS])
                ci += 1
            else:
                # tail singles: split across both rings in parallel
                half = G * TILE_ROWS // 2
                rings[0].dma_start(out_t1[start][:, :half], ob[:, :half])
                rings[1].dma_start(
                    out_t1[start][:, half : G * TILE_ROWS],
                    ob[:, half : G * TILE_ROWS],
                )


def build_nc(groups=GROUPS):
    qtot = groups * 128
    nstt = len(stt_tiles())
    nc = bacc.Bacc(
        "TRN2", target_bir_lowering=False, debug=False, num_devices=N_CORES
    )
    t_aug = nc.dram_tensor(
        "t_aug", [KC, G, qtot], F16, kind="ExternalInput"
    ).ap()
    rhs_c = nc.dram_tensor(
        "rhs_c", [S, KC, 512], F16, kind="ExternalInput"
    ).ap()
    stt_c = nc.dram_tensor(
        "stt_c", [128, 2 * D + R * nstt], F16, kind="ExternalInput"
    ).ap()
    out = nc.dram_tensor(
        "out", [groups * G * TILE_ROWS, D], F16, kind="ExternalOutput"
    ).ap()
    with tile.TileContext(nc) as tc:
        build_body(tc, out, t_aug, rhs_c, stt_c, groups, qtot)
    nc.compile()
    return nc


def _split_f16(x64):
    """hi/lo fp16 split: hi + lo ~= x to ~2^-22 rel."""
    hi = x64.astype(NPF16)
    lo = (x64 - hi.astype(np.float64)).astype(NPF16)
    return hi, lo


def affine_consts(control_points):
    """A, B ([128] float64) of the collapsed affine map out = t*A + B."""
    cp = np.asarray(control_points, dtype=np.float64)
    A = cp[1 : NUM_CP - 1].sum(axis=0) - cp[0]
    i = np.arange(1, NUM_CP - 1, dtype=np.float64)
    B = cp[0] + ((1.0 - i)[:, None] * cp[1 : NUM_CP - 1]).sum(axis=0)
    return A, B


def make_rhs(A, B):
    """Constant rhs tiles [S, KC, 512] fp16 (A diag + B_hi/B_lo rows)."""
    A_hi = A.astype(NPF16)
    B_hi, B_lo = _split_f16(B)
    rhs = np.zeros((S, KC, 512), NPF16)
    for s in range(S):
        for m in range(4):
            j = m + 4 * s
            sl = slice(128 * m, 128 * (m + 1))
            rhs[s, j, sl] = A_hi
            rhs[s, R, sl] = B_hi
            rhs[s, R + 1, sl] = B_lo
    return rhs


def make_t_aug(t_shard):
    """[KC, G, Q] fp16: slab c = t phases for rows 24*Qg + 8c + j + ones."""
    q = t_shard.shape[0] // (R * G)
    ph = t_shard.astype(NPF16).reshape(q, G, R).transpose(2, 1, 0)
    ones = np.ones((2, G, q), NPF16)
    return np.ascontiguousarray(np.concatenate([ph, ones], axis=0))


def make_stt(t_shard, A, B):
    """[128, 2D + R*nstt] fp16: A_rep | B_rep | per-STT-tile t columns."""
    stt = stt_tiles()
    out = np.zeros((128, 2 * D + R * len(stt)), NPF16)
    out[:, :D] = A.astype(NPF16)[None, :]
    out[:, D : 2 * D] = B.astype(NPF16)[None, :]
    for i, g in enumerate(stt):
        h, c = divmod(g, G)
        # column w holds t[24*(128h+q) + 8c + w] for partition q
        rows = 24 * (128 * h + np.arange(128))[:, None] + 8 * c + np.arange(R)
        out[:, 2 * D + R * i : 2 * D + R * (i + 1)] = t_shard[rows].astype(
            NPF16
        )
    return np.ascontiguousarray(out)


_NC_CACHE = {}


def _get_nc():
    if "nc" not in _NC_CACHE:
        _NC_CACHE["nc"] = build_nc()
    return _NC_CACHE["nc"]


def prepare_in_maps(t, control_points):
    t = np.asarray(t, dtype=np.float32)
    A, B = affine_consts(control_points)
    rhs = make_rhs(A, B)
    t_clipped = np.clip(t, 0.0, 1.0)
    tpad = np.zeros(NPAD, np.float32)
    tpad[: t.shape[0]] = t_clipped
    shards = tpad.reshape(N_CORES, NPC)
    return [
        {
            "t_aug": make_t_aug(shards[c]),
            "rhs_c": rhs,
            "stt_c": make_stt(shards[c], A, B),
        }
        for c in range(N_CORES)
    ]


def kernel(t, control_points):
    t = np.asarray(t)
    assert t.shape == (N_TOTAL,), t.shape
    nc = _get_nc()
    in_maps = prepare_in_maps(t, control_points)
    res = bass_utils.run_bass_kernel_spmd(
        nc, in_maps, core_ids=list(range(N_CORES))
    )
    full = np.concatenate([res.results[c]["out"] for c in range(N_CORES)], axis=0)
    return np.ascontiguousarray(full[:N_TOTAL]).astype(np.float32)


if __name__ == "__main__":
    t = np.random.default_rng(0).random(N_TOTAL, dtype=np.float32)
    cp = np.random.default_rng(1).normal(size=(NUM_CP, D)).astype(np.float32)
    out = kernel(t, cp)
    A, B = affine_consts(cp)
    expect = t.astype(np.float64)[:, None] * A[None, :] + B[None, :]
    err = np.abs(out - expect).max() / (np.abs(expect).max() + 1e-9)
    l2 = np.linalg.norm(out - expect) / np.linalg.norm(expect)
    print("self-check max rel err:", err, " l2:", l2)


# revision 21
# speedup vs baseline: 1.0376x; 1.0376x over previous
"""Trainium2 Bass kernel for nn_CubicSpline (embedding_lookup-style affine map).

Reference computes, for t in [0,1):
    w[n,i] = 1 - |t[n] - i|          (i = 0..62)
    out    = w @ cp[:63]             ([N,63] @ [63,128])

For t in [0,1] the triangular weights collapse algebraically:
    out[n,:] = t[n] * A + B
    A = sum_{i=1}^{62} cp[i] - cp[0]
    B = cp[0] + sum_{i=1}^{62} (1-i) * cp[i]

The device only needs to materialize a rank-1 affine map -- purely memory
bound on the output write. The device emits float16 (l2 rel err ~2e-4,
dominated by fp16 rounding of the result; gate is 2e-2) and the host
upcasts to float32, halving HBM write traffic, which is the roofline.

The PE on this part runs at a fixed 1.2 GHz column rate (HAM never
unthrottles), i.e. ~0.85 us per N=512 matmul, so the PE streaming of
123k psum columns (~105 us) and the 32.3 MB output DMA (~95 us) are
co-critical. Layout / engine budget per core:

  * rows grouped 24 per q-column: per-core row = 24*Qg + 8*c + j
    (Qg q-column, c chunk 0..2, j phase 0..7). Host packs
    t_aug[j, c, Qg] = fp16 t phases, rows 8/9 = ones (for B_hi/B_lo).
  * PE tiles: one K=10 weight load (lhsT = t_aug[:, c, 128h:+128]) and
    two N=512 fp16 matmuls against a constant block-diagonal rhs
    [10, 1024] (A on the phase diagonal, B_hi/B_lo on the ones rows)
    -> PSUM holds t*A + B for 1024 rows as [128 x 1024].
  * PSUM -> SBUF fp32->fp16 copies: ScalarE (3 of 5 PE tiles per
    6-block), VectorE (2 of 5).
  * every 6th tile bypasses the PE entirely: VectorE computes it with 8
    scalar_tensor_tensor ops, out[q, w*128:+128] = A_rep * t_col[q] +
    B_rep (per-partition scalar = the t value of output row 24q+8c+w).
  * copies/STT fill [128, 6144] fp16 buffers (2 groups); each buffer is
    one contiguous 1.5 MB HBM write. Output DMAs are issued ONLY from
    SyncE (SP-HWDGE) and GpSimdE (SWDGE) so their sem waits never
    head-of-line-block the compute engines' queues; the two rings
    alternate so each ring's ~2us completion stall hides under the
    other's streaming. The two final groups go as ring-parallel singles
    to shorten the tail.
"""

import os
import sys
from contextlib import ExitStack

for _p in ("/opt/trn_rl_repo", "/root/.axon_site/_ro/trn_rl_repo"):
    if os.path.isdir(_p) and _p not in sys.path:
        sys.path.insert(0, _p)

import ml_dtypes
import numpy as np

import concourse.mybir as mybir
import concourse.tile as tile
from concourse import bacc
from concourse import bass_utils

N_TOTAL = 1_000_000
D = 128
NUM_CP = 64
N_CORES = 8

R = 8                    # phase rows per chunk (= rows per partition per psum tile)
G = 3                    # chunks (psum tiles) per output group
KC = R + 2               # contraction rows: 8 fp16 t phases + 2 ones rows
S = R // 4               # N=512 matmuls per psum tile
TILE_ROWS = 128 * R      # rows per psum tile
TILES = 123              # tiles per core
GROUPS = TILES // G      # output groups (768 KB each) per core
NPC = TILES * TILE_ROWS  # rows per core
NPAD = N_CORES * NPC     # padded rows total
QTOT = NPC // (R * G)    # q-columns per core
STT_MOD = 6              # every 6th tile computed on VectorE, not the PE

F32 = mybir.dt.float32
F16 = mybir.dt.float16
NPF16 = np.float16
NPBF16 = ml_dtypes.bfloat16


def stt_tiles():
    return [g for g in range(TILES) if g % STT_MOD == STT_MOD - 1]


# GpSimd supports neither PSUM reads nor TensorScalarPtr (walrus ISA
# check), so it is DMA-issue only; ScalarE takes 4 of 5 PE-tile copies
# and VectorE takes 1 copy + all STT tiles.


def build_body(tc, out_ap, t_aug_ap, rhs_ap, stt_ap, groups, qtot):
    """Tile-framework kernel body."""
    nc = tc.nc
    mult, add = mybir.AluOpType.mult, mybir.AluOpType.add
    sttset = set(stt_tiles())
    # single-group view: [groups, 128, 3072], group h / partition q / (w d)
    # -> row 24*(128h+q) + w', col d: contiguous 6 KB per partition.
    out_t1 = out_ap.rearrange("(h q w) d -> h q (w d)", q=128, w=R * G)
    # paired view for groups 1-2 and tripled view for groups 3..38: one
    # contiguous 1.5 / 2.25 MB write per dma_start.
    grows = TILE_ROWS * G
    out_t2 = out_ap[grows : 3 * grows, :].rearrange(
        "(P p q w) d -> P q p (w d)", p=2, q=128, w=R * G
    )
    ntrip = (groups - 5) // 3
    out_t3 = out_ap[3 * grows : (3 + 3 * ntrip) * grows, :].rearrange(
        "(P p q w) d -> P q p (w d)", p=3, q=128, w=R * G
    )

    with ExitStack() as ctx:
        psum_bufs = (16 * 1024) // (TILE_ROWS * 4)  # fill the 8 PSUM banks
        tpool = ctx.enter_context(tc.tile_pool(name="tpool", bufs=1))
        cpool = ctx.enter_context(tc.tile_pool(name="cpool", bufs=1))
        opool = ctx.enter_context(tc.tile_pool(name="opool", bufs=3))
        ppool = ctx.enter_context(
            tc.tile_pool(name="ppool", bufs=psum_bufs, space="PSUM")
        )

        # DMA-issue rings: engines with no compute work, so their
        # dma_start sem-waits cannot block copies.
        rings = [nc.sync, nc.gpsimd]

        # t_aug chunks: first small chunk lands fast for the first matmul.
        ngroups = qtot // 128
        bounds = [0, 6 * 128]
        take = (ngroups - 6 + 1) // 2
        bounds.append(bounds[1] + take * 128)
        bounds.append(ngroups * 128)
        t_tiles = []
        for c in range(len(bounds) - 1):
            lo, hi = bounds[c], bounds[c + 1]
            tt = tpool.tile([KC, G, hi - lo], F16, name=f"tch{c}", tag=f"tch{c}")
            rings[c % 2].dma_start(tt[:], t_aug_ap[:, :, lo:hi])
            t_tiles.append(tt)

        # constants ride the ACT HWDGE path (ScalarE is idle until the
        # first copy, and these are ready at kernel start).
        rhs_sb = cpool.tile([KC, S * 512], F16)
        for s in range(S):
            nc.scalar.dma_start(rhs_sb[:, 512 * s : 512 * (s + 1)], rhs_ap[s])
        nstt = len(sttset)
        ab_rep = cpool.tile([128, 2 * D], F16, name="ab_rep")
        nc.scalar.dma_start(ab_rep[:], stt_ap[:, : 2 * D])
        t_stt = cpool.tile([128, R * nstt], F16, name="t_stt")
        nc.scalar.dma_start(t_stt[:], stt_ap[:, 2 * D : 2 * D + R * nstt])
        a_rep = ab_rep[:, :D]
        b_rep = ab_rep[:, D : 2 * D]

        def lhsT_for(h, c):
            col = h * 128
            for i in range(len(bounds) - 1):
                if col < bounds[i + 1]:
                    off = col - bounds[i]
                    return t_tiles[i][:, c, off : off + 128]
            raise AssertionError

        # DMA chunks: [0] [1-2] [3-5 .. 36-38] [39] [40]
        chunks = [(0, 1), (1, 2)] + [(3 + 3 * p, 3) for p in range(ntrip)]
        chunks += [(groups - 2, 1), (groups - 1, 1)]

        nthst = 0  # running STT tile index
        ci = 0     # ring rotation index
        for start, glen in chunks:
            ob = opool.tile([128, 3 * G * TILE_ROWS], F16, name="ob")
            for gi in range(glen * G):
                g = start * G + gi
                h, c = divmod(g, G)
                osl = slice(gi * TILE_ROWS, (gi + 1) * TILE_ROWS)
                if g in sttset:
                    # per phase w, the rows 24*(128h+q) + 8c + w form one
                    # partition-aligned [128, 128] block:
                    # out = A_rep * t + B_rep (no PE, no PSUM).
                    eng = nc.vector
                    for w in range(R):
                        tcol = t_stt[:, nthst * R + w : nthst * R + w + 1]
                        dsl = slice(gi * TILE_ROWS + w * D,
                                    gi * TILE_ROWS + (w + 1) * D)
                        eng.scalar_tensor_tensor(
                            ob[:, dsl], a_rep, tcol, b_rep, mult, add
                        )
                    nthst += 1
                    continue
                psum = ppool.tile([128, TILE_ROWS], F32, name="psum")
                lhsT = lhsT_for(h, c)
                for s in range(S):
                    sl = slice(512 * s, 512 * (s + 1))
                    nc.tensor.matmul(
                        psum[:, sl], lhsT, rhs_sb[:, sl], start=True, stop=True
                    )
                if g % STT_MOD in (1, 3):
                    nc.vector.tensor_copy(ob[:, osl], psum[:])
                else:
                    nc.scalar.copy(ob[:, osl], psum[:])
            if glen == 3:
                dst = out_t3[(start - 3) // 3]
                src = ob[:].rearrange("q (p f) -> q p f", p=3)
                rings[ci % 2].dma_start(dst, src)
                ci += 1
            elif glen == 2:
                dst = out_t2[0]
                src = ob[:, : 2 * G * TILE_ROWS].rearrange(
                    "q (p f) -> q p f", p=2
                )
                rings[ci % 2].dma_start(dst, src)
                ci += 1
            elif start == 0:
                rings[ci % 2].dma_start(out_t1[0], ob[:, : G * TILE_ROWS])
                ci += 1
            else:
                # tail singles: split across both rings in parallel
                half = G * TILE_ROWS // 2
                rings[0].dma_start(out_t1[start][:, :half], ob[:, :half])
                rings[1].dma_start(
                    out_t1[start][:, half : G * TILE_ROWS],
                    ob[:, half : G * TILE_ROWS],
                )


def build_nc(groups=GROUPS):
    qtot = groups * 128
    nstt = len(stt_tiles())
    nc = bacc.Bacc(
        "TRN2", target_bir_lowering=False, debug=False, num_devices=N_CORES
    )
    t_aug = nc.dram_tensor(
        "t_aug", [KC, G, qtot], F16, kind="ExternalInput"
    ).ap()
    rhs_c = nc.dram_tensor(
        "rhs_c", [S, KC, 512], F16, kind="ExternalInput"
    ).ap()
    stt_c = nc.dram_tensor(
        "stt_c", [128, 2 * D + R * nstt], F16, kind="ExternalInput"
    ).ap()
    out = nc.dram_tensor(
        "out", [groups * G * TILE_ROWS, D], F16, kind="ExternalOutput"
    ).ap()
    with tile.TileContext(nc) as tc:
        build_body(tc, out, t_aug, rhs_c, stt_c, groups, qtot)
    nc.compile()
    return nc


def _split_f16(x64):
    """hi/lo fp16 split: hi + lo ~= x to ~2^-22 rel."""
    hi = x64.astype(NPF16)
    lo = (x64 - hi.astype(np.float64)).astype(NPF16)
    return hi, lo


def affine_consts(control_points):
    """A, B ([128] float64) of the collapsed affine map out = t*A + B."""
    cp = np.asarray(control_points, dtype=np.float64)
    A = cp[1 : NUM_CP - 1].sum(axis=0) - cp[0]
    i = np.arange(1, NUM_CP - 1, dtype=np.float64)
    B = cp[0] + ((1.0 - i)[:, None] * cp[1 : NUM_CP - 1]).sum(axis=0)
    return A, B


def make_rhs(A, B):
    """Constant rhs tiles [S, KC, 512] fp16 (A diag + B_hi/B_lo rows)."""
    A_hi = A.astype(NPF16)
    B_hi, B_lo = _split_f16(B)
    rhs = np.zeros((S, KC, 512), NPF16)
    for s in range(S):
        for m in range(4):
            j = m + 4 * s
            sl = slice(128 * m, 128 * (m + 1))
            rhs[s, j, sl] = A_hi
            rhs[s, R, sl] = B_hi
            rhs[s, R + 1, sl] = B_lo
    return rhs


def make_t_aug(t_shard):
    """[KC, G, Q] fp16: slab c = t phases for rows 24*Qg + 8c + j + ones."""
    q = t_shard.shape[0] // (R * G)
    ph = t_shard.astype(NPF16).reshape(q, G, R).transpose(2, 1, 0)
    ones = np.ones((2, G, q), NPF16)
    return np.ascontiguousarray(np.concatenate([ph, ones], axis=0))


def make_stt(t_shard, A, B):
    """[128, 2D + R*nstt] fp16: A_rep | B_rep | per-STT-tile t columns."""
    stt = stt_tiles()
    out = np.zeros((128, 2 * D + R * len(stt)), NPF16)
    out[:, :D] = A.astype(NPF16)[None, :]
    out[:, D : 2 * D] = B.astype(NPF16)[None, :]
    for i, g in enumerate(stt):
        h, c = divmod(g, G)
        # column w holds t[24*(128h+q) + 8c + w] for partition q
        rows = 24 * (128 * h + np.arange(128))[:, None] + 8 * c + np.arange(R)
        out[:, 2 * D + R * i : 2 * D + R * (i + 1)] = t_shard[rows].astype(
            NPF16
        )
    return np.ascontiguousarray(out)


_NC_CACHE = {}


def _get_nc():
    if "nc" not in _NC_CACHE:
        _NC_CACHE["nc"] = build_nc()
    return _NC_CACHE["nc"]


def prepare_in_maps(t, control_points):
    t = np.asarray(t, dtype=np.float32)
    A, B = affine_consts(control_points)
    rhs = make_rhs(A, B)
    t_clipped = np.clip(t, 0.0, 1.0)
    tpad = np.zeros(NPAD, np.float32)
    tpad[: t.shape[0]] = t_clipped
    shards = tpad.reshape(N_CORES, NPC)
    return [
        {
            "t_aug": make_t_aug(shards[c]),
            "rhs_c": rhs,
            "stt_c": make_stt(shards[c], A, B),
        }
        for c in range(N_CORES)
    ]


def kernel(t, control_points):
    t = np.asarray(t)
    assert t.shape == (N_TOTAL,), t.shape
    nc = _get_nc()
    in_maps = prepare_in_maps(t, control_points)
    res = bass_utils.run_bass_kernel_spmd(
        nc, in_maps, core_ids=list(range(N_CORES))
    )
    full = np.concatenate([res.results[c]["out"] for c in range(N_CORES)], axis=0)
    return np.ascontiguousarray(full[:N_TOTAL]).astype(np.float32)


if __name__ == "__main__":
    t = np.random.default_rng(0).random(N_TOTAL, dtype=np.float32)
    cp = np.random.default_rng(1).normal(size=(NUM_CP, D)).astype(np.float32)
    out = kernel(t, cp)
    A, B = affine_consts(cp)
    expect = t.astype(np.float64)[:, None] * A[None, :] + B[None, :]
    err = np.abs(out - expect).max() / (np.abs(expect).max() + 1e-9)
    l2 = np.linalg.norm(out - expect) / np.linalg.norm(expect)
    print("self-check max rel err:", err, " l2:", l2)
